# revision 8
# baseline (speedup 1.0000x reference)
"""GQA causal attention (B=4, S=2048, D=2048, H=16, KVH=8, HD=128) on 8 trn2 cores.

Sharding: batch x head-group. Core c = (b, g) with b = c // 2, g = c % 2.
Host sums the two partials per batch (row-sharded wo => partial sums).

v4: fp8 DoubleRow 3-term projections. Every projection matmul (Q/K/V/out)
runs as fp8e4m3 DoubleRow over k-pairs (2 k-tiles of 128 per matmul, 0.5
cycles/row in the cost model) with two e5m2 residual-correction terms:

    y = x8 @ w8 + x8 @ dw8 + dx8 @ w8        (dropped dx8@dw8 ~ 0.04%)

where x8 = e4m3(x), dx8 = e5m2(x - x8), w8 = e4m3(64 w), dw8 = e5m2(64w - w8).
All three terms share one PSUM accumulation group (same 64x scale, undone at
eviction / folded into rope tables). Host precomputes and packs all splits.
Attention core (scores, exp, PV, softmax) stays bf16 as in v3.

Emission structure is v3's software pipeline: projection work is interleaved
into attention as PE filler so the in-order PE never waits on ACT's exp.
"""

import numpy as np

D = 2048
S = 2048
HQ = 8        # q heads per core
HKV = 4       # kv heads per core
HD = 128
KK2 = D // 256        # 8 contraction k-pairs (2 x 128 each)
QC = S // 512         # 4 sequence chunks of 512
NST = S // 128        # 16 sequence tiles of 128
SCALE = 1.0 / float(np.sqrt(HD))
WS = 64.0             # weight pre-scale (power of 2; keeps fp8 out of subnormals)

_CACHE = {}
_SENTINEL = object()


def _swap_mask():
    m = []
    for i in range(16):
        m += [2 * i + 1, 2 * i]
    return m


def build_nc():
    from contextlib import ExitStack

    import concourse.mybir as mybir
    import concourse.tile as tile
    from concourse import bacc

    f32 = mybir.dt.float32
    bf16 = mybir.dt.bfloat16
    e4 = mybir.dt.float8e4
    e5 = mybir.dt.float8e5
    AF = mybir.ActivationFunctionType
    OP = mybir.AluOpType
    DR = mybir.MatmulPerfMode.DoubleRow

    nc = bacc.Bacc(None, target_bir_lowering=False)

    X8 = nc.dram_tensor("X8", [128, KK2 * QC * 1024], e4, kind="ExternalInput")
    DX8 = nc.dram_tensor("DX8", [128, KK2 * QC * 1024], e5, kind="ExternalInput")
    WQ8 = nc.dram_tensor("WQ8", [128, KK2 * 2048], e4, kind="ExternalInput")
    DWQ8 = nc.dram_tensor("DWQ8", [128, KK2 * 2048], e5, kind="ExternalInput")
    WK8 = nc.dram_tensor("WK8", [128, KK2 * 1024], e4, kind="ExternalInput")
    DWK8 = nc.dram_tensor("DWK8", [128, KK2 * 1024], e5, kind="ExternalInput")
    WV8 = nc.dram_tensor("WV8", [128, KK2 * 1024], e4, kind="ExternalInput")
    DWV8 = nc.dram_tensor("DWV8", [128, KK2 * 1024], e5, kind="ExternalInput")
    WO8 = nc.dram_tensor("WO8", [128, 4 * 4096], e4, kind="ExternalInput")
    DWO8 = nc.dram_tensor("DWO8", [128, 4 * 4096], e5, kind="ExternalInput")
    cosb = nc.dram_tensor("cosb", [HD, S], bf16, kind="ExternalInput")
    sinb = nc.dram_tensor("sinb", [HD, S], bf16, kind="ExternalInput")
    out = nc.dram_tensor("out", [S, D], bf16, kind="ExternalOutput")

    SWAP = _swap_mask()

    with tile.TileContext(nc) as tc, ExitStack() as ctx:
        constp = ctx.enter_context(tc.tile_pool(name="constp", bufs=1))
        tabs = ctx.enter_context(tc.tile_pool(name="tabs", bufs=1))
        kvp = ctx.enter_context(tc.tile_pool(name="kvp", bufs=1))
        vstp = ctx.enter_context(tc.tile_pool(name="vstp", bufs=1))
        wresp = ctx.enter_context(tc.tile_pool(name="wresp", bufs=1))
        xsp = ctx.enter_context(tc.tile_pool(name="xsp", bufs=8))
        dxsp = ctx.enter_context(tc.tile_pool(name="dxsp", bufs=8))
        tmpp = ctx.enter_context(tc.tile_pool(name="tmpp", bufs=2))
        qtp = ctx.enter_context(tc.tile_pool(name="qtp", bufs=8))
        ptp = ctx.enter_context(tc.tile_pool(name="ptp", bufs=3))
        onp = ctx.enter_context(tc.tile_pool(name="onp", bufs=12))
        donp = ctx.enter_context(tc.tile_pool(name="donp", bufs=12))
        onbp = ctx.enter_context(tc.tile_pool(name="onbp", bufs=1))
        onup = ctx.enter_context(tc.tile_pool(name="onup", bufs=2))
        rdp = ctx.enter_context(tc.tile_pool(name="rdp", bufs=4))
        rbp = ctx.enter_context(tc.tile_pool(name="rbp", bufs=2))
        rtp = ctx.enter_context(tc.tile_pool(name="rtp", bufs=2))
        oevp = ctx.enter_context(tc.tile_pool(name="oevp", bufs=4))
        psP = ctx.enter_context(tc.tile_pool(name="psP", bufs=4, space="PSUM"))
        psS = ctx.enter_context(tc.tile_pool(name="psS", bufs=2, space="PSUM"))
        psO = ctx.enter_context(tc.tile_pool(name="psO", bufs=1, space="PSUM"))
        psD = ctx.enter_context(tc.tile_pool(name="psD", bufs=1, space="PSUM"))

        ones1 = constp.tile([128, 1], e4, name="ones1")
        nc.vector.memset(ones1[:], 1.0)
        biast = constp.tile([128, 1], f32, name="biast")
        nc.vector.memset(biast[:], -float(np.log(8.0)))
        idf = constp.tile([128, 128], f32, name="idf")
        nc.vector.memset(idf[:], 1.0)
        nc.gpsimd.affine_select(
            out=idf[:], in_=idf[:], compare_op=OP.is_equal, fill=0.0,
            base=0, pattern=[[1, 128]], channel_multiplier=-1,
        )

        # ---- resident weights (fp8 pair tiles), loaded once ---------------
        WQp = [wresp.tile([128, 2, 1024], e4, name=f"wq{k}") for k in range(KK2)]
        DWQp = [wresp.tile([128, 2, 1024], e5, name=f"dwq{k}") for k in range(KK2)]
        WKp = [wresp.tile([128, 2, 512], e4, name=f"wk{k}") for k in range(KK2)]
        DWKp = [wresp.tile([128, 2, 512], e5, name=f"dwk{k}") for k in range(KK2)]
        WVp = [wresp.tile([128, 2, 512], e4, name=f"wv{k}") for k in range(KK2)]
        DWVp = [wresp.tile([128, 2, 512], e5, name=f"dwv{k}") for k in range(KK2)]
        WOp = [wresp.tile([128, 2, 2048], e4, name=f"wo{j}") for j in range(4)]
        DWOp = [wresp.tile([128, 2, 2048], e5, name=f"dwo{j}") for j in range(4)]
        ct_full = tabs.tile([HD, S], bf16, name="cos_full")
        st_full = tabs.tile([HD, S], bf16, name="sin_full")

        KT = [kvp.tile([HD, S], bf16, name=f"kt{h}") for h in range(HKV)]
        # combined V tile per st-pair: dim1 = (v8_s0, v8_s1, dv8_s0,
        # dv8_s1); same dtype so a step-2 slice pairs (v8, dv8) of one slot
        # for the fused diagonal-sliver DoubleRow matmul.
        VC = [vstp.tile([128, 4, HKV * HD], e4, name=f"v{p}")
              for p in range(NST // 2)]

        def gen_load_x(sc, xa, dxa):
            """Emits next-chunk x8/dx8 loads lazily (after the last reader of
            the previous chunk's tiles in emission order)."""
            for kp in range(KK2):
                t = xsp.tile([128, 2, 512], e4, name="xs")
                off = (kp * QC + sc) * 1024
                nc.sync.dma_start(t[:, :, :], X8[:, off:off + 1024])
                xa.append(t)
            for kp in range(KK2):
                t = dxsp.tile([128, 2, 512], e5, name="dxs")
                off = (kp * QC + sc) * 1024
                nc.sync.dma_start(t[:, :, :], DX8[:, off:off + 1024])
                dxa.append(t)
            yield

        # Startup loads, round-robined across the three DGE queues (SP and
        # ACT HWDGE + Pool SWDGE, which are all idle before attention starts)
        # in need-ordered groups, so no single ~1.2us-cadence queue paces the
        # dense chunk-0 projections.
        xa0, dxa0 = [], []
        for kp in range(KK2):
            nc.sync.dma_start(WKp[kp][:, :, :], WK8[:, kp * 1024:(kp + 1) * 1024])
            t = xsp.tile([128, 2, 512], e4, name="xs")
            q = nc.scalar if kp < 2 else nc.sync
            q.dma_start(t[:, :, :], X8[:, kp * QC * 1024:kp * QC * 1024 + 1024])
            xa0.append(t)
        for kp in range(KK2):
            nc.scalar.dma_start(DWKp[kp][:, :, :],
                                DWK8[:, kp * 1024:(kp + 1) * 1024])
            t = dxsp.tile([128, 2, 512], e5, name="dxs")
            nc.scalar.dma_start(t[:, :, :],
                                DX8[:, kp * QC * 1024:kp * QC * 1024 + 1024])
            dxa0.append(t)
        for kp in range(KK2):
            nc.gpsimd.dma_start(WVp[kp][:, :, :], WV8[:, kp * 1024:(kp + 1) * 1024])
            nc.gpsimd.dma_start(DWVp[kp][:, :, :],
                                DWV8[:, kp * 1024:(kp + 1) * 1024])
        nc.scalar.dma_start(ct_full[:, 0:512], cosb[:, 0:512])
        nc.scalar.dma_start(st_full[:, 0:512], sinb[:, 0:512])
        for kp in range(KK2):
            nc.sync.dma_start(WQp[kp][:, :, :], WQ8[:, kp * 2048:(kp + 1) * 2048])
            nc.sync.dma_start(DWQp[kp][:, :, :], DWQ8[:, kp * 2048:(kp + 1) * 2048])
        for j in range(4):
            nc.sync.dma_start(WOp[j][:, :, :], WO8[:, j * 4096:(j + 1) * 4096])
            nc.sync.dma_start(DWOp[j][:, :, :], DWO8[:, j * 4096:(j + 1) * 4096])
        for s4 in range(1, 4):
            nc.sync.dma_start(ct_full[:, s4 * 512:(s4 + 1) * 512],
                              cosb[:, s4 * 512:(s4 + 1) * 512])
            nc.sync.dma_start(st_full[:, s4 * 512:(s4 + 1) * 512],
                              sinb[:, s4 * 512:(s4 + 1) * 512])

        def evict_v(st, psv_ap):
            """psum (64x scale) -> v8 (e4, true scale) + e4 residual."""
            p, hf = st // 2, st % 2
            with nc.allow_low_precision(reason="fp8 V split; residual "
                                        "corrects it"):
                nc.vector.tensor_scalar_mul(VC[p][:, hf, :], psv_ap, 1.0 / WS)
                nc.vector.scalar_tensor_tensor(
                    out=VC[p][:, 2 + hf, :], in0=psv_ap, scalar=1.0 / WS,
                    in1=VC[p][:, hf, :], op0=OP.mult, op1=OP.subtract)

        def rope(psrc, sc, dst):
            # psum holds 64x-scaled q/k; tables are host-pre-scaled by 1/64.
            ssl = slice(sc * 512, (sc + 1) * 512)
            t1 = tmpp.tile([128, 512], f32, name="rope_t1")
            nc.scalar.copy(t1[:], psrc[:])
            sw = tmpp.tile([128, 512], f32, name="rope_sw")
            nc.vector.stream_shuffle(sw[:], t1[:], SWAP)
            nc.vector.tensor_mul(sw[:], sw[:], st_full[:, ssl])
            nc.vector.tensor_mul(t1[:], t1[:], ct_full[:, ssl])
            nc.vector.tensor_add(dst, t1[:], sw[:])

        # ---- 3-term DoubleRow matmul helper -------------------------------
        # term 0: w8 @ x8, term 1: dw8 @ x8, term 2: w8 @ dx8. All into one
        # PSUM group (start at t0/kp0, stop at t2/kp_last).
        def dr3(ps_ap, t, kp, wt, dwt, xt, dxt, start, stop):
            w = dwt if t == 1 else wt
            xx = dxt if t == 2 else xt
            nc.tensor.matmul(ps_ap, w, xx, start=start, stop=stop, perf_mode=DR)

        # ---- filler generators -------------------------------------------
        def gen_kproj(sc, xa, dxa):
            """K projection (2 passes of 2 kv heads) + rope into KT."""
            ssl = slice(sc * 512, (sc + 1) * 512)
            for pa in range(2):
                pss = [psP.tile([128, 512], f32, name="psP") for _ in range(2)]
                for t in range(3):
                    for kp in range(KK2):
                        for i in range(2):
                            h = pa * 2 + i
                            dr3(pss[i][:],
                                t, kp,
                                WKp[kp][:, :, h * HD:(h + 1) * HD],
                                DWKp[kp][:, :, h * HD:(h + 1) * HD],
                                xa[kp][:, :, :], dxa[kp][:, :, :],
                                start=(t == 0 and kp == 0),
                                stop=(t == 2 and kp == KK2 - 1))
                        yield
                for i in range(2):
                    rope(pss[i], sc, KT[pa * 2 + i][:, ssl])

        def gen_vproj(sc, xa, dxa):
            """V projection (2 passes of 2 seq tiles), evicted bf16 (x1/64)."""
            for pa in range(2):
                psv = [psP.tile([128, 512], f32, name="psP") for _ in range(2)]
                for t in range(3):
                    for kp in range(KK2):
                        for i in range(2):
                            st = pa * 2 + i
                            w = xa[kp] if t != 2 else dxa[kp]
                            mv = WVp[kp] if t != 1 else DWVp[kp]
                            nc.tensor.matmul(
                                psv[i][:], w[:, :, st * 128:(st + 1) * 128],
                                mv[:, :, :],
                                start=(t == 0 and kp == 0),
                                stop=(t == 2 and kp == KK2 - 1), perf_mode=DR)
                        yield
                for i in range(2):
                    evict_v(sc * 4 + pa * 2 + i, psv[i][:])

        def gen_qproj(sc, xa, dxa, pairs, QTr):
            """Q projection for the given head pairs + rope into QTr."""
            for pa in pairs:
                psq = [psP.tile([128, 512], f32, name="psP") for _ in pa]
                for t in range(3):
                    for kp in range(KK2):
                        for i, h in enumerate(pa):
                            c0 = h * HD
                            dr3(psq[i][:],
                                t, kp,
                                WQp[kp][:, :, c0:c0 + HD],
                                DWQp[kp][:, :, c0:c0 + HD],
                                xa[kp][:, :, :], dxa[kp][:, :, :],
                                start=(t == 0 and kp == 0),
                                stop=(t == 2 and kp == KK2 - 1))
                        yield
                for i, h in enumerate(pa):
                    qt = qtp.tile([128, 512], bf16, name="qt")
                    rope(psq[i], sc, qt[:])
                    QTr[h] = qt

        def gen_outproj(sc, ON8c, DON8c, extra_bank=False):
            """Output projection for chunk sc (3-term DR over head-pairs).

            extra_bank: rotate a third PSUM bank (psD) into the group cycle —
            only valid after the last attention chunk has retired psD."""
            idx = 0
            for dc in range(4):
                dsl = slice(dc * 512, (dc + 1) * 512)
                for qs in range(4):
                    pool = psP
                    if extra_bank and idx % 3 == 2:
                        pool = psD if (idx // 3) % 2 == 0 else psO
                    ps = pool.tile([128, 512], f32,
                                   name={id(psD): "psD", id(psO): "psO",
                                         id(psP): "psP"}[id(pool)])
                    idx += 1
                    n = 0
                    # j-major: a group emitted right after the last attention
                    # chunk can start on the early head-pairs while the late
                    # heads' normalization tails are still draining.
                    for j in range(4):
                        for t in range(3):
                            w = ON8c[j] if t != 2 else DON8c[j]
                            mv = WOp[j] if t != 1 else DWOp[j]
                            nc.tensor.matmul(
                                ps[:], w[:, :, qs * 128:(qs + 1) * 128],
                                mv[:, :, dsl],
                                start=(t == 0 and j == 0),
                                stop=(t == 2 and j == 3), perf_mode=DR)
                            n += 1
                            if n % 2 == 0:
                                yield
                    oev = oevp.tile([128, 512], bf16, name="oev")
                    nc.vector.tensor_scalar_mul(oev[:], ps[:], 1.0 / WS)
                    r0 = sc * 512 + qs * 128
                    nc.sync.dma_start(out[r0:r0 + 128, dsl], oev[:])
                    yield

        def chain(*gens):
            for g in gens:
                yield from g

        # ---- attention with interleaved filler ---------------------------
        def emit_attn(sc, QTr, fillers, n_fill, skew=False):
            """Attention for chunk sc, pulling from `fillers` between kt steps.

            +1 software pipeline: PV/denominator MMs for kt run after the
            scores MM of kt+1, giving exp(kt) a full PE quantum to finish.
            """
            nk = 4 * (sc + 1)
            n_att = HQ * nk
            ON8c = [onp.tile([128, 2, 512], e4, name="on8") for _ in range(4)]
            DON8c = [donp.tile([128, 2, 512], e5, name="don8") for _ in range(4)]
            dps = psD.tile([128, 512], f32, name="psD")
            fill_acc = 0.0
            fill_per = 1.1 * n_fill / n_att
            done_fill = 0
            quantum = [0]

            def pull_fill(n):
                nonlocal done_fill
                for _ in range(n):
                    if next(fillers, _SENTINEL) is _SENTINEL:
                        return
                    done_fill += 1

            tailq = []

            def run_tailq():
                for item in list(tailq):
                    item[0] -= 1
                    if item[0] <= 0:
                        item[1]()
                        tailq.remove(item)

            for h in range(HQ):
                # head h's scores need QTr[h]; when its projection is itself a
                # filler (chunk 0 head-group 1), drain fillers until it lands.
                while QTr[h] is None:
                    before = done_fill
                    pull_fill(1)
                    if done_fill == before:
                        raise RuntimeError("fillers exhausted before QTr ready")
                kvh = h // 2
                po = psO.tile([128, 512], f32, name="psO")
                pend = []  # [(kt, off, nj, pair_tile)] exp'd, PV pending
                cur_pair = None
                first_pv = True
                for kt in range(nk + 1):
                    if kt < nk:
                        j = kt - 4 * sc
                        off = 128 * j if j > 0 else 0
                        nj = 512 - off
                        ss = psS.tile([128, 512], f32, name="psS")
                        nc.tensor.matmul(
                            ss[:, :nj],
                            KT[kvh][:, kt * 128:(kt + 1) * 128],
                            QTr[h][:, off:],
                            start=True, stop=True,
                        )
                        if kt % 2 == 0:
                            cur_pair = ptp.tile([128, 2, 512], e4, name="pt")
                        par = kt % 2
                        # pt8 = exp(s)/8 in e4m3 (bias keeps the tail under
                        # the 240 max); the /8 cancels in the normalization.
                        nc.scalar.activation(cur_pair[:, par, off:off + nj],
                                             ss[:, :nj], AF.Exp,
                                             scale=SCALE, bias=biast[:])
                        if j >= 0:
                            # only the diagonal 128-col block can violate
                            # causality; the rest is strictly below diagonal.
                            nc.gpsimd.affine_select(
                                out=cur_pair[:, par, off:off + 128],
                                in_=cur_pair[:, par, off:off + 128],
                                compare_op=OP.is_ge, fill=0.0, base=0,
                                pattern=[[1, 128]], channel_multiplier=-1,
                            )
                        pend.append((kt, off, nj, cur_pair))
                    # PV for pair (k0,k1) once scores(k1+1) has been issued:
                    # one DoubleRow mm over both kt slots + e5 V-residual mm;
                    # a non-DR sliver covers k0's extra diagonal columns.
                    if len(pend) >= 4 or (kt >= nk and len(pend) >= 2):
                        (k0, off0, nj0, pt0), (k1, off1, nj1, pt1) = pend[:2]
                        pend = pend[2:]
                        p = k0 // 2
                        offc = off1
                        last_pair = (k1 == nk - 1)
                        vsl = slice(kvh * HD, (kvh + 1) * HD)
                        nc.tensor.matmul(
                            po[:, offc:], VC[p][:, 0:2, vsl], pt0[:, :, offc:],
                            start=first_pv, stop=False, perf_mode=DR)
                        first_pv = False
                        nc.tensor.matmul(
                            po[:, offc:], VC[p][:, 2:4, vsl], pt0[:, :, offc:],
                            start=False, stop=(last_pair and off0 == offc),
                            perf_mode=DR)
                        if off0 < offc:
                            # fused sliver: (v8_s0, dv8_s0) slots against the
                            # same pt columns (stride-0 slot broadcast).
                            nw = offc - off0
                            nc.tensor.matmul(
                                po[:, off0:offc], VC[p][:, 0:4:2, vsl],
                                pt0[:, 0, off0:offc].rearrange(
                                    "p (x c) -> p x c", x=1
                                ).to_broadcast([128, 2, nw]),
                                start=False, stop=last_pair, perf_mode=DR)
                        for kk_, aoff, anj in ((k0, off0, nj0), (k1, off1, nj1)):
                            pj = kk_ - 4 * sc
                            par = kk_ % 2
                            for qb in range(4):
                                if pj > qb:
                                    continue
                                dcol = h * 64 + qb * 16 + kk_
                                nc.tensor.matmul(
                                    dps[:, dcol:dcol + 1],
                                    pt0[:, par, qb * 128:(qb + 1) * 128],
                                    ones1[:],
                                    start=True,
                                    stop=True,
                                )
                    quantum[0] += 1
                    rate = fill_per
                    if skew:
                        # back-load fillers: the late steps of the last chunk
                        # are exp-paced on ACT and can absorb more PE work.
                        rate *= 1.4 if quantum[0] < n_att // 2 else 0.6
                    fill_acc += rate
                    nf = int(fill_acc)
                    fill_acc -= nf
                    pull_fill(nf)
                    run_tailq()

                # head tail: evict po unnormalized (frees the PSUM bank),
                # then normalization chain off the critical path.
                onu = onup.tile([128, 512], bf16, name="onu")
                nc.vector.tensor_copy(onu[:], po[:])
                rdpre = rdp.tile([128, 4], f32, name="rdpre")
                for qb in range(4):
                    a = h * 64 + qb * 16
                    nc.vector.reduce_sum(
                        rdpre[:, qb:qb + 1],
                        dps[:, a:a + 4 * sc + qb + 1],
                        axis=mybir.AxisListType.X,
                    )
                rd = rdp.tile([128, 4], bf16, name="rd")
                with nc.allow_low_precision(reason="bf16 1/denominator; 0.4% "
                                            "rounding is inside tolerance"):
                    nc.vector.reciprocal(rd[:], rdpre[:])
                rdrow = rtp.tile([1, 512], bf16, name="rdrow")
                if h == HQ - 1:
                    # Last head gates the next out-projection: low-latency
                    # PE-transpose path for the reciprocal row.
                    rd7 = rdp.tile([128, 4], f32, name="rd7")
                    nc.vector.reciprocal(rd7[:], rdpre[:])
                    for qb in range(4):
                        nc.tensor.transpose(
                            dps[0:1, qb * 128:(qb + 1) * 128],
                            rd7[:, qb:qb + 1], idf[:],
                        )
                    nc.vector.tensor_copy(rdrow[0:1, :], dps[0:1, 0:512])
                else:
                    for qb in range(4):
                        nc.sync.dma_start(rdrow[0:1, qb * 128:(qb + 1) * 128],
                                          rd[:, qb:qb + 1])

                def tail(h=h, rdrow=rdrow, onu=onu):
                    rb = rbp.tile([128, 512], bf16, name="rb")
                    nc.gpsimd.partition_broadcast(rb[:, :], rdrow[0:1, :],
                                                  channels=128)
                    onb = onbp.tile([128, 512], bf16, name="onb")
                    nc.vector.tensor_mul(onb[:], onu[:], rb[:])
                    jj, par = h // 2, h % 2
                    with nc.allow_low_precision(reason="fp8 split of attn out; "
                                                "e5m2 residual corrects it"):
                        nc.vector.tensor_copy(ON8c[jj][:, par, :], onb[:])
                        nc.vector.tensor_sub(DON8c[jj][:, par, :], onb[:],
                                             ON8c[jj][:, par, :])

                if h == HQ - 1:
                    tail()
                else:
                    # long deferral: the broadcast's rdrow input needs ~2.5us
                    # to land; issuing the broadcast early would head-of-line
                    # block the (in-order) Pool engine, starving the
                    # affine_selects that gate PV pairs. ON tiles are not
                    # consumed until two windows later, so the slack is free.
                    tailq.append([10, tail])

            while tailq:
                run_tailq()
            pull_fill(10 ** 9)
            return ON8c, DON8c

        # ---- main pipeline ------------------------------------------------
        # chunk-0 projections run dense up front on all 8 PSUM banks.
        QTr = [None] * HQ

        def bank_tiles(pools):
            names = {id(psP): "psP", id(psS): "psS", id(psO): "psO",
                     id(psD): "psD"}
            return [p.tile([128, 512], f32, name=names[id(p)]) for p in pools]

        pss = bank_tiles([psP, psP, psS, psS])
        for t in range(3):
            for kp in range(KK2):
                for h in range(HKV):
                    dr3(pss[h][:], t, kp,
                        WKp[kp][:, :, h * HD:(h + 1) * HD],
                        DWKp[kp][:, :, h * HD:(h + 1) * HD],
                        xa0[kp][:, :, :], dxa0[kp][:, :, :],
                        start=(t == 0 and kp == 0),
                        stop=(t == 2 and kp == KK2 - 1))
        for h in range(HKV):
            rope(pss[h], 0, KT[h][:, 0:512])
        psv = bank_tiles([psO, psD, psP, psP])
        for t in range(3):
            for kp in range(KK2):
                for st in range(4):
                    w = xa0[kp] if t != 2 else dxa0[kp]
                    mv = WVp[kp] if t != 1 else DWVp[kp]
                    nc.tensor.matmul(psv[st][:], w[:, :, st * 128:(st + 1) * 128],
                                     mv[:, :, :],
                                     start=(t == 0 and kp == 0),
                                     stop=(t == 2 and kp == KK2 - 1),
                                     perf_mode=DR)
        for st in range(4):
            evict_v(st, psv[st][:])
        # heads 0-1 dense (head 0 gates the first attention scores
        # matmul); heads 2-7 are deferred into the attention-0 filler
        # stream to shorten the DMA-bound startup.
        for _ in gen_qproj(0, xa0, dxa0, [(0, 1)], QTr):
            pass

        # Out-projections are deferred two windows (oc -> window oc+2,
        # with outproj(1) and (2) both in window 3): the late attention
        # windows are exp-paced on ACT, so they can absorb the most PE
        # filler; the early windows are already PE-saturated.
        xa_by_sc = {0: (xa0, dxa0)}
        ON_by_sc = {}
        for sc in range(QC):
            # head-group-1 q-projection of the CURRENT chunk leads the filler
            # chain: for sc=0 it gates heads 4-7 of this very chunk's
            # attention; for sc>0 it was deferred from the previous window.
            parts = [gen_qproj(sc, *xa_by_sc[sc],
                                [(2, 3), (4, 5), (6, 7)] if sc == 0
                                else [(4, 5), (6, 7)], QTr)]
            n_fill = 72 if sc == 0 else 48
            if sc + 1 < QC:
                xa_next, dxa_next = [], []
                xa_by_sc[sc + 1] = (xa_next, dxa_next)
                parts.append(gen_load_x(sc + 1, xa_next, dxa_next))
                parts.append(gen_kproj(sc + 1, xa_next, dxa_next))
                QTr_next = [None] * HQ
                parts.append(gen_qproj(sc + 1, xa_next, dxa_next,
                                       [(0, 1), (2, 3)], QTr_next))
                parts.append(gen_vproj(sc + 1, xa_next, dxa_next))
                n_fill += 145
            ocs = {2: [0], 3: [1, 2]}.get(sc, [])
            for oc in ocs:
                parts.append(gen_outproj(oc, *ON_by_sc[oc]))
                n_fill += 112
            ON_by_sc[sc] = emit_attn(sc, QTr, chain(*parts), n_fill,
                                     skew=False)
            if sc + 1 < QC:
                QTr = QTr_next

        for _ in gen_outproj(QC - 1, *ON_by_sc[QC - 1], extra_bank=True):
            pass

    nc.compile()
    return nc


def _get_nc():
    if "nc" not in _CACHE:
        _CACHE["nc"] = build_nc()
    return _CACHE["nc"]


def _host_prep(x, wq, wk, wv, wo, pos_cos, pos_sin):
    import ml_dtypes

    ne4 = ml_dtypes.float8_e4m3
    ne5 = ml_dtypes.float8_e5m2

    x = np.asarray(x, dtype=np.float32)
    wq = np.asarray(wq, dtype=np.float32)
    wk = np.asarray(wk, dtype=np.float32)
    wv = np.asarray(wv, dtype=np.float32)
    wo = np.asarray(wo, dtype=np.float32)
    pos_cos = np.asarray(pos_cos, dtype=np.float32)
    pos_sin = np.asarray(pos_sin, dtype=np.float32)

    import ml_dtypes as _md
    cosb = (np.repeat(pos_cos.T, 2, axis=0) / WS).astype(_md.bfloat16).copy()
    sinb = np.repeat(pos_sin.T, 2, axis=0)
    sinb[0::2, :] *= -1.0
    sinb = (sinb / WS).astype(_md.bfloat16).copy()

    def split84(a):
        hi = a.astype(ne4)
        lo = (a - hi.astype(np.float32)).astype(ne5)
        return hi, lo

    def pack_w(w, ncols):
        # [D_k, ncols] -> [128, KK2 * 2 * ncols]: pair tile kp holds k-tiles
        # (2kp, 2kp+1) as [p][i*ncols + m].
        kk2 = w.shape[0] // 256
        r = w.reshape(kk2, 2, 128, ncols).transpose(2, 0, 1, 3)
        return np.ascontiguousarray(r.reshape(128, kk2 * 2 * ncols))

    def pack_x(xt):
        # xT [D, S] -> [128, KK2 * QC * 1024]: tile (kp, sc) at
        # (kp*QC+sc)*1024, layout [p][i*512 + c].
        r = xt.reshape(KK2, 2, 128, QC, 512).transpose(2, 0, 3, 1, 4)
        return np.ascontiguousarray(r.reshape(128, KK2 * QC * 1024))

    in_maps = []
    for c in range(8):
        b, g = c // 2, c % 2
        xt = np.ascontiguousarray(x[b].T)
        x8, dx8 = split84(xt)
        wq8, dwq8 = split84(WS * wq[:, g * 1024:(g + 1) * 1024])
        wk8, dwk8 = split84(WS * wk[:, g * 512:(g + 1) * 512])
        wv8, dwv8 = split84(WS * wv[:, g * 512:(g + 1) * 512])
        wo8, dwo8 = split84(WS * wo[g * 1024:(g + 1) * 1024, :])
        in_maps.append({
            "X8": pack_x(x8), "DX8": pack_x(dx8),
            "WQ8": pack_w(wq8, 1024), "DWQ8": pack_w(dwq8, 1024),
            "WK8": pack_w(wk8, 512), "DWK8": pack_w(dwk8, 512),
            "WV8": pack_w(wv8, 512), "DWV8": pack_w(dwv8, 512),
            "WO8": pack_w(wo8, 2048), "DWO8": pack_w(dwo8, 2048),
            "cosb": cosb,
            "sinb": sinb,
        })
    return in_maps


def kernel(x, wq, wk, wv, wo, pos_cos, pos_sin):
    from concourse.bass_utils import run_bass_kernel_spmd

    nc = _get_nc()
    in_maps = _host_prep(x, wq, wk, wv, wo, pos_cos, pos_sin)
    res = run_bass_kernel_spmd(nc, in_maps, core_ids=list(range(8)))
    outs = [np.asarray(r["out"], dtype=np.float32) for r in res.results]
    full = np.empty((4, S, D), dtype=np.float32)
    for b in range(4):
        full[b] = outs[2 * b] + outs[2 * b + 1]
    return full
